# revision 17
# baseline (speedup 1.0000x reference)
"""Trainium2 Bass kernel for nn_Head_75118978007668.

Computes, for x:[B,S,D], concept_map(cm):[D,D,D] (B=4, S=2048, D=128):
    s[b,t] = sum_{j<t} lam^(t-j) x[b,j]          (lam = 1/1.2 decayed prefix sum)
    out[b,t,f] = sum_{d,e} x[b,t,d] * s[b,t,e] * cm[f,d,e]

Sharding: 8 cores, each owns 1024 contiguous positions of one batch row
(4 rows x 2 halves).  The scan carry across the half-split is recovered
exactly (to fp32) from a 256-position halo (lam^256 ~ 4.5e-21 << fp32 eps).

Per-core dataflow (positions tiled 8 x 128):
  - carries + s tiles: small PE matmuls (as before), s stored [p,t,e,1] fp32
  - main per tile: Y = xT_t.T @ W2 in N=512 matmuls (bf16), grouped as
      * 11 "pair" groups (8 e's = 2 matmuls -> one 2-bank PSUM tile):
        DVE 1024-wide broadcast mult  Z = Y * s[.,e-range]  (stride-0 AP),
        GPSIMD 1024-wide bf16 add onto an 8-lane accumulator
      * 10 "c" chunks (4 e's = 1 matmul -> 1-bank PSUM tile):
        4x ACT scaled copies (scale=s) -> bf16, DVE 512-wide add onto a
        4-lane accumulator
  - lane-fold per tile (DVE bf16 tree) -> acc[p,t,f] fp32 -> DMA out
  where W2[d, e*128+f] = cm[f, d, e]  (host-transposed).
"""

import numpy as np
import ml_dtypes

import concourse.bass as bass
import concourse.tile as tile
from concourse import bacc, mybir
from concourse.bass import ds, ts
from concourse.bass_utils import run_bass_kernel_spmd

B, S, D = 4, 2048, 128
NCORES = 8
CHUNK = S // 2          # positions per core (1024)
NT = CHUNK // 128       # position tiles per core (8)
P = 128
HALO = 256
F32 = mybir.dt.float32
BF16 = mybir.dt.bfloat16

# match the reference's fp32 constant 1.2 exactly
LAM = 1.0 / np.float64(np.float32(1.2))

# per tile: NPAIR pair-groups of 8 e's (DVE mult path, 2 pairs fill one
# 16-lane "unit") + NC chunks of 4 e's (ACT path, 4 chunks fill one unit).
NPAIR = 11
NC = 10
assert NPAIR * 8 + NC * 4 == 128  # u5 is mixed: pair 10 + chunks 0,1
NUNIT = 8                         # 8 tree leaves of [P, 16, P]
# level-1 tree adds executed on GPSIMD (rest of tree + folds on DVE)
GP_L1 = 4
GP_L2 = 0

_CACHE = {}
LAST_RESULTS = None


def _host_constants():
    k = np.arange(P, dtype=np.float64)
    i = k
    LT = np.where(i[:, None] < k[None, :], LAM ** (k[None, :] - i[:, None]), 0.0)
    powv = (LAM ** k)[None, :]                      # [1, 128]
    vw = (LAM ** (P - i))[:, None]                  # [128, 1]
    j = np.arange(HALO, dtype=np.float64)
    hw = (LAM ** (HALO - j)).reshape(2, P).T        # [128, 2]
    t = np.arange(NT, dtype=np.float64)
    M9 = np.zeros((NT, NT + 1), dtype=np.float64)
    M9[:, 0] = LAM ** (P * t)
    for tt in range(NT):
        for jj in range(tt):
            M9[tt, jj + 1] = LAM ** (P * (tt - 1 - jj))
    LT9 = M9.T                                      # [9, 8]
    f32 = np.float32
    return {
        "lt": LT.astype(f32),
        "powv": powv.astype(f32),
        "vw": vw.astype(f32),
        "hw": hw.astype(f32),
        "lt9": LT9.astype(f32),
    }


def _emit_order():
    """Interleave NC c-chunks among NPAIR pair-groups for pipelining."""
    order = []
    ic, ip = 0, 0
    acc = 0.0
    for _ in range(NPAIR + NC):
        acc += NC / (NPAIR + NC)
        if acc >= 1.0 and ic < NC:
            order.append(("c", ic)); ic += 1; acc -= 1.0
        elif ip < NPAIR:
            order.append(("p", ip)); ip += 1
        else:
            order.append(("c", ic)); ic += 1
    return order


def _tree_pairs(n):
    """Pairings for a binary reduction of n leaves: list of levels, each a
    list of (i, j) index pairs into the previous level's list."""
    levels = []
    cur = n
    while cur > 1:
        pairs = [(2 * i, 2 * i + 1) for i in range(cur // 2)]
        odd = cur % 2
        levels.append((pairs, odd))
        cur = cur // 2 + odd
    return levels


def _build_nc():
    nc = bacc.Bacc("TRN2", target_bir_lowering=False, debug=False,
                   num_devices=NCORES)
    x_d = nc.declare_dram_parameter("x", [P, NT, P], F32, isOutput=False)
    xt_d = nc.declare_dram_parameter("xt", [P, CHUNK], BF16, isOutput=False)
    halo_d = nc.declare_dram_parameter("halo", [P, 2, P], F32, isOutput=False)
    w2_d = nc.declare_dram_parameter("w2", [P, P * P], BF16, isOutput=False)
    lt_d = nc.declare_dram_parameter("lt", [P, P], F32, isOutput=False)
    pow_d = nc.declare_dram_parameter("powv", [1, P], F32, isOutput=False)
    vw_d = nc.declare_dram_parameter("vw", [P, 1], F32, isOutput=False)
    hw_d = nc.declare_dram_parameter("hw", [P, 2], F32, isOutput=False)
    lt9_d = nc.declare_dram_parameter("lt9", [NT + 1, NT], F32, isOutput=False)
    out_d = nc.declare_dram_parameter("out", [P, NT, P], F32, isOutput=True)

    mult = mybir.AluOpType.mult
    add = mybir.AluOpType.add
    order = _emit_order()

    with tile.TileContext(nc) as tc:
        with tc.tile_pool(name="consts", bufs=1) as consts:
            w2_sb = [consts.tile([P, 2048], BF16, name=f"w2_sb{i}")
                     for i in range(8)]
            xt_sb = consts.tile([P, CHUNK], BF16)
            x_sb = consts.tile([P, NT, P], F32)
            halo_sb = consts.tile([P, 2, P], F32)
            lt_sb = consts.tile([P, P], F32)
            pow_sb = consts.tile([1, P], F32)
            vw_sb = consts.tile([P, 1], F32)
            hw_sb = consts.tile([P, 2], F32)
            lt9_sb = consts.tile([NT + 1, NT], F32)
            v9_sb = consts.tile([NT + 1, P], F32)
            c0_sb = consts.tile([1, P], F32)
            va_sb = consts.tile([1, 4 * P], F32)
            vb_sb = consts.tile([1, 4 * P], F32)
            c8_sb = consts.tile([NT, P], F32)
            c_all = consts.tile([1, NT * P], F32)
            s_sb = consts.tile([P, NT, P, 1], F32)   # [p, t, e, 1]
            acc = consts.tile([P, NT, P], F32)       # [p, t, f] final

            for i in range(8):
                nc.sync.dma_start(out=w2_sb[i][:, :],
                                  in_=w2_d[:, ds(2048 * i, 2048)])
            nc.sync.dma_start(out=xt_sb[:, :], in_=xt_d[:, :])
            nc.sync.dma_start(out=x_sb[:, :, :], in_=x_d[:, :, :])
            nc.sync.dma_start(out=halo_sb[:, :, :], in_=halo_d[:, :, :])
            nc.sync.dma_start(out=lt_sb[:, :], in_=lt_d[:, :])
            nc.sync.dma_start(out=pow_sb[:, :], in_=pow_d[:, :])
            nc.sync.dma_start(out=vw_sb[:, :], in_=vw_d[:, :])
            nc.sync.dma_start(out=hw_sb[:, :], in_=hw_d[:, :])
            nc.sync.dma_start(out=lt9_sb[:, :], in_=lt9_d[:, :])

            # ---- carries ----
            with tc.tile_pool(name="psum_c", bufs=1, space="PSUM") as psum_c:
                c0_ps = psum_c.tile([1, P], F32)
                nc.tensor.matmul(c0_ps[:, :], lhsT=hw_sb[:, 0:1],
                                 rhs=halo_sb[:, 0, :], start=True, stop=False)
                nc.tensor.matmul(c0_ps[:, :], lhsT=hw_sb[:, 1:2],
                                 rhs=halo_sb[:, 1, :], start=False, stop=True)
                vps_a = psum_c.tile([1, 4 * P], F32, tag="vps_a")
                vps_b = psum_c.tile([1, 4 * P], F32, tag="vps_b")
                nc.tensor.matmul(vps_a[:, :], lhsT=vw_sb[:, :],
                                 rhs=x_sb[:, 0:4, :], start=True, stop=True)
                nc.tensor.matmul(vps_b[:, :], lhsT=vw_sb[:, :],
                                 rhs=x_sb[:, 4:8, :], start=True, stop=True)
                nc.vector.tensor_copy(c0_sb[:, :], c0_ps[:, :])
                nc.vector.tensor_copy(va_sb[:, :], vps_a[:, :])
                nc.vector.tensor_copy(vb_sb[:, :], vps_b[:, :])
                nc.sync.dma_start(out=v9_sb[0:1, :], in_=c0_sb[:, :])
                nc.sync.dma_start(out=v9_sb[1:5, :], in_=va_sb[:, :])
                nc.sync.dma_start(out=v9_sb[5:9, :], in_=vb_sb[:, :])
                c_ps = psum_c.tile([NT, P], F32, tag="c_ps")
                nc.tensor.matmul(c_ps[:, :], lhsT=lt9_sb[:, :],
                                 rhs=v9_sb[:, :], start=True, stop=True)
                nc.vector.tensor_copy(c8_sb[:, :], c_ps[:, :])
                nc.sync.dma_start(out=c_all[:, :], in_=c8_sb[:, :])

            # ---- s tiles: s = L @ x_t + pow (x) c_t  -> [p,t,e,1] ----
            with tc.tile_pool(name="psum_s", bufs=2, space="PSUM") as psum_s:
                for t in range(NT):
                    sp = psum_s.tile([P, P, 1], F32)
                    nc.tensor.matmul(sp[:, :, :], lhsT=lt_sb[:, :],
                                     rhs=x_sb[:, t, :], start=True, stop=False)
                    nc.tensor.matmul(sp[:, :, :], lhsT=pow_sb[:, :],
                                     rhs=c_all[:, ts(t, P)], start=False, stop=True)
                    nc.vector.tensor_copy(s_sb[:, t, :, :], sp[:, :, :])

            # ---- main (software-pipelined: tree of tile t-1 is emitted
            # after the units of tile t, so DVE/GP never wait in-tile) ----
            with tc.tile_pool(name="psum_p", bufs=3, space="PSUM") as psum_p, \
                 tc.tile_pool(name="psum_cc", bufs=2, space="PSUM") as psum_cc, \
                 tc.tile_pool(name="unit_ring", bufs=18) as unit_ring, \
                 tc.tile_pool(name="l1_ring", bufs=2) as l1_ring, \
                 tc.tile_pool(name="l2_ring", bufs=2) as l2_ring, \
                 tc.tile_pool(name="fold_ring", bufs=2) as fold_ring:

                def emit_units(t):
                    xt_t = xt_sb[:, ts(t, P)]
                    units = [unit_ring.tile([P, 16, P], BF16, name="unit",
                                            tag="unit")
                             for k in range(NUNIT)]
                    for kind, idx in order:
                        if kind == "p":
                            e0 = 8 * idx
                            yq = psum_p.tile([P, 8, P], F32, name="yq",
                                             tag="yq")
                            for h in range(2):
                                cset = (e0 + 4 * h) // 4
                                nc.tensor.matmul(
                                    yq[:, ds(4 * h, 4), :], lhsT=xt_t,
                                    rhs=w2_sb[cset // 4][:, ds(512 * (cset % 4), 512)],
                                    start=True, stop=True)
                            u = units[idx // 2]
                            half = idx % 2
                            sb_b = s_sb[:, t, ds(e0, 8), :].broadcast_to([P, 8, P])
                            nc.vector.tensor_tensor(
                                out=u[:, ds(8 * half, 8), :], in0=yq[:, :, :],
                                in1=sb_b, op=mult)
                        else:
                            e0 = 8 * NPAIR + 4 * idx
                            cset = e0 // 4
                            yc = psum_cc.tile([P, 4, P], F32, name="yc",
                                              tag="yc")
                            nc.tensor.matmul(
                                yc[:, :, :], lhsT=xt_t,
                                rhs=w2_sb[cset // 4][:, ds(512 * (cset % 4), 512)],
                                start=True, stop=True)
                            if idx < 2:
                                u = units[5]
                                q0 = 8 + 4 * idx
                            else:
                                u = units[6 + (idx - 2) // 4]
                                q0 = 4 * ((idx - 2) % 4)
                            for jj in range(4):
                                e = e0 + jj
                                nc.scalar.activation(
                                    u[:, q0 + jj, :], yc[:, jj, :],
                                    mybir.ActivationFunctionType.Copy,
                                    scale=s_sb[:, t, e, :])
                    return units

                def emit_tree(t, units):
                    # DVE subtree over its own units u0..u4 (1024-wide
                    # halves, chained); GP consumes its unit u5 plus the
                    # ACT units u6,u7, then folds everything.
                    va = l2_ring.tile([P, 16, P], BF16, name="va", tag="va")
                    for h in range(2):
                        sl = ds(8 * h, 8)
                        nc.vector.tensor_tensor(
                            out=va[:, sl, :], in0=units[0][:, sl, :],
                            in1=units[1][:, sl, :], op=add)
                        for k in (2, 3, 4):  # u5 belongs to GP side
                            nc.vector.tensor_tensor(
                                out=va[:, sl, :], in0=units[k][:, sl, :],
                                in1=va[:, sl, :], op=add)
                    f8 = fold_ring.tile([P, 8, P], BF16, name="f8", tag="f8")
                    nc.vector.tensor_tensor(
                        out=f8[:, :, :], in0=va[:, 0:8, :],
                        in1=va[:, 8:16, :], op=add)

                    g = l1_ring.tile([P, 16, P], BF16, name="g", tag="g")
                    nc.gpsimd.tensor_tensor(out=g[:, :, :],
                                            in0=units[5][:, :, :],
                                            in1=units[6][:, :, :], op=add)
                    nc.gpsimd.tensor_tensor(out=g[:, :, :],
                                            in0=units[7][:, :, :],
                                            in1=g[:, :, :], op=add)
                    g8 = fold_ring.tile([P, 8, P], BF16, name="g8", tag="g8")
                    nc.gpsimd.tensor_tensor(
                        out=g8[:, :, :], in0=g[:, 0:8, :],
                        in1=g[:, 8:16, :], op=add)
                    m8 = fold_ring.tile([P, 8, P], BF16, name="m8", tag="m8")
                    nc.gpsimd.tensor_tensor(
                        out=m8[:, :, :], in0=f8[:, :, :], in1=g8[:, :, :],
                        op=add)
                    f4 = fold_ring.tile([P, 4, P], BF16, name="f4", tag="f4")
                    nc.gpsimd.tensor_tensor(
                        out=f4[:, :, :], in0=m8[:, 0:4, :], in1=m8[:, 4:8, :],
                        op=add)
                    f2 = fold_ring.tile([P, 2, P], BF16, name="f2", tag="f2")
                    nc.gpsimd.tensor_tensor(
                        out=f2[:, :, :], in0=f4[:, 0:2, :], in1=f4[:, 2:4, :],
                        op=add)
                    nc.gpsimd.tensor_tensor(
                        out=acc[:, t, :], in0=f2[:, 0, :], in1=f2[:, 1, :],
                        op=add)

                prev = None
                for t in range(NT):
                    u = emit_units(t)
                    if prev is not None:
                        emit_tree(t - 1, prev)
                    prev = u
                emit_tree(NT - 1, prev)

            nc.sync.dma_start(out=out_d[:, :, :], in_=acc[:, :, :])
    nc.finalize()
    return nc


def _get_nc():
    if "nc" not in _CACHE:
        _CACHE["nc"] = _build_nc()
    return _CACHE["nc"]


def kernel(x, concept_map, _trace=False):
    global LAST_RESULTS
    x = np.asarray(x, dtype=np.float32)
    cm = np.asarray(concept_map, dtype=np.float32)
    assert x.shape == (B, S, D) and cm.shape == (D, D, D)

    consts = _host_constants()
    w2 = np.ascontiguousarray(
        np.transpose(cm, (1, 2, 0)).reshape(D, D * D)).astype(ml_dtypes.bfloat16)

    in_maps = []
    for core in range(NCORES):
        b, half = divmod(core, 2)
        lo = half * CHUNK
        xc = x[b, lo:lo + CHUNK]
        x_il = np.ascontiguousarray(
            xc.reshape(NT, P, D).transpose(1, 0, 2))
        xt = np.ascontiguousarray(xc.T).astype(ml_dtypes.bfloat16)
        if half == 0:
            halo = np.zeros((P, 2, D), dtype=np.float32)
        else:
            h = x[b, lo - HALO:lo]
            halo = np.ascontiguousarray(h.reshape(2, P, D).transpose(1, 0, 2))
        in_maps.append({
            "x": x_il, "xt": xt, "halo": halo, "w2": w2, **consts,
        })

    nc = _get_nc()
    res = run_bass_kernel_spmd(nc, in_maps, list(range(NCORES)), trace=_trace)
    LAST_RESULTS = res

    out = np.empty((B, S, D), dtype=np.float32)
    for core in range(NCORES):
        b, half = divmod(core, 2)
        o = res.results[core]["out"]
        out[b, half * CHUNK:(half + 1) * CHUNK] = (
            o.transpose(1, 0, 2).reshape(CHUNK, D))
    return out


# revision 18
# speedup vs baseline: 1.3342x; 1.3342x over previous
"""Trainium2 Bass kernel for nn_Head_75118978007668.

Computes, for x:[B,S,D], concept_map(cm):[D,D,D] (B=4, S=2048, D=128):
    s[b,t] = sum_{j<t} lam^(t-j) x[b,j]          (lam = 1/1.2 decayed prefix sum)
    out[b,t,f] = sum_{d,e} x[b,t,d] * s[b,t,e] * cm[f,d,e]

Sharding: 8 cores, each owns 1024 contiguous positions of one batch row
(4 rows x 2 halves).  The scan carry across the half-split is recovered
exactly (to fp32) from a 256-position halo (lam^256 ~ 4.5e-21 << fp32 eps).

Per-core dataflow (positions tiled 8 x 128):
  - carries + s tiles: small PE matmuls (as before), s stored [p,t,e,1] fp32
  - main per tile: Y = xT_t.T @ W2 in N=512 matmuls (bf16), grouped as
      * 11 "pair" groups (8 e's = 2 matmuls -> one 2-bank PSUM tile):
        DVE 1024-wide broadcast mult  Z = Y * s[.,e-range]  (stride-0 AP),
        GPSIMD 1024-wide bf16 add onto an 8-lane accumulator
      * 10 "c" chunks (4 e's = 1 matmul -> 1-bank PSUM tile):
        4x ACT scaled copies (scale=s) -> bf16, DVE 512-wide add onto a
        4-lane accumulator
  - lane-fold per tile (DVE bf16 tree) -> acc[p,t,f] fp32 -> DMA out
  where W2[d, e*128+f] = cm[f, d, e]  (host-transposed).
"""

import numpy as np
import ml_dtypes

import concourse.bass as bass
import concourse.tile as tile
from concourse import bacc, mybir
from concourse.bass import ds, ts
from concourse.bass_utils import run_bass_kernel_spmd

B, S, D = 4, 2048, 128
NCORES = 8
CHUNK = S // 2          # positions per core (1024)
NT = CHUNK // 128       # position tiles per core (8)
P = 128
HALO = 256
F32 = mybir.dt.float32
BF16 = mybir.dt.bfloat16

# match the reference's fp32 constant 1.2 exactly
LAM = 1.0 / np.float64(np.float32(1.2))

# per tile: NPAIR pair-groups of 8 e's (DVE mult path, 2 pairs fill one
# 16-lane "unit") + NC chunks of 4 e's (ACT path, 4 chunks fill one unit).
NPAIR = 10
NC = 12
assert NPAIR * 8 + NC * 4 == 128  # u5 is mixed: pair 10 + chunks 0,1
NUNIT = 8                         # 8 tree leaves of [P, 16, P]
# level-1 tree adds executed on GPSIMD (rest of tree + folds on DVE)
GP_L1 = 4
GP_L2 = 0

_CACHE = {}
LAST_RESULTS = None


def _host_constants():
    k = np.arange(P, dtype=np.float64)
    i = k
    LT = np.where(i[:, None] < k[None, :], LAM ** (k[None, :] - i[:, None]), 0.0)
    powv = (LAM ** k)[None, :]                      # [1, 128]
    vw = (LAM ** (P - i))[:, None]                  # [128, 1]
    j = np.arange(HALO, dtype=np.float64)
    hw = (LAM ** (HALO - j)).reshape(2, P).T        # [128, 2]
    t = np.arange(NT, dtype=np.float64)
    M9 = np.zeros((NT, NT + 1), dtype=np.float64)
    M9[:, 0] = LAM ** (P * t)
    for tt in range(NT):
        for jj in range(tt):
            M9[tt, jj + 1] = LAM ** (P * (tt - 1 - jj))
    LT9 = M9.T                                      # [9, 8]
    f32 = np.float32
    return {
        "lt": LT.astype(f32),
        "powv": powv.astype(f32),
        "vw": vw.astype(f32),
        "hw": hw.astype(f32),
        "lt9": LT9.astype(f32),
    }


def _emit_order():
    """Interleave NC c-chunks among NPAIR pair-groups for pipelining."""
    order = []
    ic, ip = 0, 0
    acc = 0.0
    for _ in range(NPAIR + NC):
        acc += NC / (NPAIR + NC)
        if acc >= 1.0 and ic < NC:
            order.append(("c", ic)); ic += 1; acc -= 1.0
        elif ip < NPAIR:
            order.append(("p", ip)); ip += 1
        else:
            order.append(("c", ic)); ic += 1
    return order


def _tree_pairs(n):
    """Pairings for a binary reduction of n leaves: list of levels, each a
    list of (i, j) index pairs into the previous level's list."""
    levels = []
    cur = n
    while cur > 1:
        pairs = [(2 * i, 2 * i + 1) for i in range(cur // 2)]
        odd = cur % 2
        levels.append((pairs, odd))
        cur = cur // 2 + odd
    return levels


def _build_nc():
    nc = bacc.Bacc("TRN2", target_bir_lowering=False, debug=False,
                   num_devices=NCORES)
    x_d = nc.declare_dram_parameter("x", [P, NT, P], F32, isOutput=False)
    xt_d = nc.declare_dram_parameter("xt", [P, CHUNK], BF16, isOutput=False)
    halo_d = nc.declare_dram_parameter("halo", [P, 2, P], F32, isOutput=False)
    w2_d = nc.declare_dram_parameter("w2", [P, P * P], BF16, isOutput=False)
    lt_d = nc.declare_dram_parameter("lt", [P, P], F32, isOutput=False)
    pow_d = nc.declare_dram_parameter("powv", [1, P], F32, isOutput=False)
    vw_d = nc.declare_dram_parameter("vw", [P, 1], F32, isOutput=False)
    hw_d = nc.declare_dram_parameter("hw", [P, 2], F32, isOutput=False)
    lt9_d = nc.declare_dram_parameter("lt9", [NT + 1, NT], F32, isOutput=False)
    out_d = nc.declare_dram_parameter("out", [P, NT, P], F32, isOutput=True)

    mult = mybir.AluOpType.mult
    add = mybir.AluOpType.add
    order = _emit_order()

    with tile.TileContext(nc) as tc:
        with tc.tile_pool(name="consts", bufs=1) as consts:
            w2_sb = [consts.tile([P, 2048], BF16, name=f"w2_sb{i}")
                     for i in range(8)]
            xt_sb = consts.tile([P, CHUNK], BF16)
            x_sb = consts.tile([P, NT, P], F32)
            halo_sb = consts.tile([P, 2, P], F32)
            lt_sb = consts.tile([P, P], F32)
            pow_sb = consts.tile([1, P], F32)
            vw_sb = consts.tile([P, 1], F32)
            hw_sb = consts.tile([P, 2], F32)
            lt9_sb = consts.tile([NT + 1, NT], F32)
            v9_sb = consts.tile([NT + 1, P], F32)
            c0_sb = consts.tile([1, P], F32)
            va_sb = consts.tile([1, 4 * P], F32)
            vb_sb = consts.tile([1, 4 * P], F32)
            c8_sb = consts.tile([NT, P], F32)
            c_all = consts.tile([1, NT * P], F32)
            s_sb = consts.tile([P, NT, P, 1], F32)   # [p, t, e, 1]
            acc = consts.tile([P, NT, P], F32)       # [p, t, f] final

            for i in range(8):
                nc.sync.dma_start(out=w2_sb[i][:, :],
                                  in_=w2_d[:, ds(2048 * i, 2048)])
            nc.sync.dma_start(out=xt_sb[:, :], in_=xt_d[:, :])
            nc.sync.dma_start(out=x_sb[:, :, :], in_=x_d[:, :, :])
            nc.sync.dma_start(out=halo_sb[:, :, :], in_=halo_d[:, :, :])
            nc.sync.dma_start(out=lt_sb[:, :], in_=lt_d[:, :])
            nc.sync.dma_start(out=pow_sb[:, :], in_=pow_d[:, :])
            nc.sync.dma_start(out=vw_sb[:, :], in_=vw_d[:, :])
            nc.sync.dma_start(out=hw_sb[:, :], in_=hw_d[:, :])
            nc.sync.dma_start(out=lt9_sb[:, :], in_=lt9_d[:, :])

            # ---- carries ----
            with tc.tile_pool(name="psum_c", bufs=1, space="PSUM") as psum_c:
                c0_ps = psum_c.tile([1, P], F32)
                nc.tensor.matmul(c0_ps[:, :], lhsT=hw_sb[:, 0:1],
                                 rhs=halo_sb[:, 0, :], start=True, stop=False)
                nc.tensor.matmul(c0_ps[:, :], lhsT=hw_sb[:, 1:2],
                                 rhs=halo_sb[:, 1, :], start=False, stop=True)
                vps_a = psum_c.tile([1, 4 * P], F32, tag="vps_a")
                vps_b = psum_c.tile([1, 4 * P], F32, tag="vps_b")
                nc.tensor.matmul(vps_a[:, :], lhsT=vw_sb[:, :],
                                 rhs=x_sb[:, 0:4, :], start=True, stop=True)
                nc.tensor.matmul(vps_b[:, :], lhsT=vw_sb[:, :],
                                 rhs=x_sb[:, 4:8, :], start=True, stop=True)
                nc.vector.tensor_copy(c0_sb[:, :], c0_ps[:, :])
                nc.vector.tensor_copy(va_sb[:, :], vps_a[:, :])
                nc.vector.tensor_copy(vb_sb[:, :], vps_b[:, :])
                nc.sync.dma_start(out=v9_sb[0:1, :], in_=c0_sb[:, :])
                nc.sync.dma_start(out=v9_sb[1:5, :], in_=va_sb[:, :])
                nc.sync.dma_start(out=v9_sb[5:9, :], in_=vb_sb[:, :])
                c_ps = psum_c.tile([NT, P], F32, tag="c_ps")
                nc.tensor.matmul(c_ps[:, :], lhsT=lt9_sb[:, :],
                                 rhs=v9_sb[:, :], start=True, stop=True)
                nc.vector.tensor_copy(c8_sb[:, :], c_ps[:, :])
                nc.sync.dma_start(out=c_all[:, :], in_=c8_sb[:, :])

            # ---- s tiles: s = L @ x_t + pow (x) c_t  -> [p,t,e,1] ----
            with tc.tile_pool(name="psum_s", bufs=2, space="PSUM") as psum_s:
                for t in range(NT):
                    sp = psum_s.tile([P, P, 1], F32)
                    nc.tensor.matmul(sp[:, :, :], lhsT=lt_sb[:, :],
                                     rhs=x_sb[:, t, :], start=True, stop=False)
                    nc.tensor.matmul(sp[:, :, :], lhsT=pow_sb[:, :],
                                     rhs=c_all[:, ts(t, P)], start=False, stop=True)
                    nc.vector.tensor_copy(s_sb[:, t, :, :], sp[:, :, :])

            # ---- main (software-pipelined: tree of tile t-1 is emitted
            # after the units of tile t, so DVE/GP never wait in-tile) ----
            with tc.tile_pool(name="psum_p", bufs=3, space="PSUM") as psum_p, \
                 tc.tile_pool(name="psum_cc", bufs=2, space="PSUM") as psum_cc, \
                 tc.tile_pool(name="unit_ring", bufs=18) as unit_ring, \
                 tc.tile_pool(name="l1_ring", bufs=2) as l1_ring, \
                 tc.tile_pool(name="l2_ring", bufs=2) as l2_ring, \
                 tc.tile_pool(name="fold_ring", bufs=2) as fold_ring:

                def emit_units(t):
                    xt_t = xt_sb[:, ts(t, P)]
                    units = [unit_ring.tile([P, 16, P], BF16, name="unit",
                                            tag="unit")
                             for k in range(NUNIT)]
                    for kind, idx in order:
                        if kind == "p":
                            e0 = 8 * idx
                            yq = psum_p.tile([P, 8, P], F32, name="yq",
                                             tag="yq")
                            for h in range(2):
                                cset = (e0 + 4 * h) // 4
                                nc.tensor.matmul(
                                    yq[:, ds(4 * h, 4), :], lhsT=xt_t,
                                    rhs=w2_sb[cset // 4][:, ds(512 * (cset % 4), 512)],
                                    start=True, stop=True)
                            u = units[idx // 2]
                            half = idx % 2
                            sb_b = s_sb[:, t, ds(e0, 8), :].broadcast_to([P, 8, P])
                            nc.vector.tensor_tensor(
                                out=u[:, ds(8 * half, 8), :], in0=yq[:, :, :],
                                in1=sb_b, op=mult)
                        else:
                            e0 = 8 * NPAIR + 4 * idx
                            cset = e0 // 4
                            yc = psum_cc.tile([P, 4, P], F32, name="yc",
                                              tag="yc")
                            nc.tensor.matmul(
                                yc[:, :, :], lhsT=xt_t,
                                rhs=w2_sb[cset // 4][:, ds(512 * (cset % 4), 512)],
                                start=True, stop=True)
                            u = units[NPAIR // 2 + idx // 4]
                            q0 = 4 * (idx % 4)
                            for jj in range(4):
                                e = e0 + jj
                                nc.scalar.activation(
                                    u[:, q0 + jj, :], yc[:, jj, :],
                                    mybir.ActivationFunctionType.Copy,
                                    scale=s_sb[:, t, e, :])
                    return units

                def emit_tree(t, units):
                    # GP chain over units[0:4] (2048-wide, aliased ok on GP);
                    # DVE chain over units[4:8] (1024-wide halves, ping-pong).
                    g = l1_ring.tile([P, 16, P], BF16, name="g", tag="g")
                    nc.gpsimd.tensor_tensor(out=g[:, :, :],
                                            in0=units[0][:, :, :],
                                            in1=units[1][:, :, :], op=add)
                    nc.gpsimd.tensor_tensor(out=g[:, :, :],
                                            in0=units[2][:, :, :],
                                            in1=g[:, :, :], op=add)
                    nc.gpsimd.tensor_tensor(out=g[:, :, :],
                                            in0=units[3][:, :, :],
                                            in1=g[:, :, :], op=add)
                    va = l2_ring.tile([P, 16, P], BF16, name="va", tag="va")
                    vb = l2_ring.tile([P, 16, P], BF16, name="vb", tag="vb")
                    for h in range(2):
                        sl = ds(8 * h, 8)
                        nc.vector.tensor_tensor(
                            out=va[:, sl, :], in0=units[4][:, sl, :],
                            in1=units[5][:, sl, :], op=add)
                        nc.vector.tensor_tensor(
                            out=vb[:, sl, :], in0=units[6][:, sl, :],
                            in1=units[7][:, sl, :], op=add)
                        nc.vector.tensor_tensor(
                            out=va[:, sl, :], in0=va[:, sl, :],
                            in1=vb[:, sl, :], op=add)
                    # cross add + lane fold (all DVE, 1024-wide max)
                    f8 = fold_ring.tile([P, 8, P], BF16, name="f8", tag="f8")
                    nc.vector.tensor_tensor(
                        out=f8[:, :, :], in0=va[:, 0:8, :],
                        in1=va[:, 8:16, :], op=add)
                    g8 = fold_ring.tile([P, 8, P], BF16, name="g8", tag="g8")
                    nc.vector.tensor_tensor(
                        out=g8[:, :, :], in0=g[:, 0:8, :],
                        in1=g[:, 8:16, :], op=add)
                    m8 = fold_ring.tile([P, 8, P], BF16, name="m8", tag="m8")
                    nc.vector.tensor_tensor(
                        out=m8[:, :, :], in0=f8[:, :, :], in1=g8[:, :, :],
                        op=add)
                    f4 = fold_ring.tile([P, 4, P], BF16, name="f4", tag="f4")
                    nc.vector.tensor_tensor(
                        out=f4[:, :, :], in0=m8[:, 0:4, :], in1=m8[:, 4:8, :],
                        op=add)
                    f2 = fold_ring.tile([P, 2, P], BF16, name="f2", tag="f2")
                    nc.vector.tensor_tensor(
                        out=f2[:, :, :], in0=f4[:, 0:2, :], in1=f4[:, 2:4, :],
                        op=add)
                    nc.vector.tensor_tensor(
                        out=acc[:, t, :], in0=f2[:, 0, :], in1=f2[:, 1, :],
                        op=add)

                prev = None
                for t in range(NT):
                    u = emit_units(t)
                    if prev is not None:
                        emit_tree(t - 1, prev)
                    prev = u
                emit_tree(NT - 1, prev)

            nc.sync.dma_start(out=out_d[:, :, :], in_=acc[:, :, :])
    nc.finalize()
    return nc


def _get_nc():
    if "nc" not in _CACHE:
        _CACHE["nc"] = _build_nc()
    return _CACHE["nc"]


def kernel(x, concept_map, _trace=False):
    global LAST_RESULTS
    x = np.asarray(x, dtype=np.float32)
    cm = np.asarray(concept_map, dtype=np.float32)
    assert x.shape == (B, S, D) and cm.shape == (D, D, D)

    consts = _host_constants()
    w2 = np.ascontiguousarray(
        np.transpose(cm, (1, 2, 0)).reshape(D, D * D)).astype(ml_dtypes.bfloat16)

    in_maps = []
    for core in range(NCORES):
        b, half = divmod(core, 2)
        lo = half * CHUNK
        xc = x[b, lo:lo + CHUNK]
        x_il = np.ascontiguousarray(
            xc.reshape(NT, P, D).transpose(1, 0, 2))
        xt = np.ascontiguousarray(xc.T).astype(ml_dtypes.bfloat16)
        if half == 0:
            halo = np.zeros((P, 2, D), dtype=np.float32)
        else:
            h = x[b, lo - HALO:lo]
            halo = np.ascontiguousarray(h.reshape(2, P, D).transpose(1, 0, 2))
        in_maps.append({
            "x": x_il, "xt": xt, "halo": halo, "w2": w2, **consts,
        })

    nc = _get_nc()
    res = run_bass_kernel_spmd(nc, in_maps, list(range(NCORES)), trace=_trace)
    LAST_RESULTS = res

    out = np.empty((B, S, D), dtype=np.float32)
    for core in range(NCORES):
        b, half = divmod(core, 2)
        o = res.results[core]["out"]
        out[b, half * CHUNK:(half + 1) * CHUNK] = (
            o.transpose(1, 0, 2).reshape(CHUNK, D))
    return out
